# revision 23
# baseline (speedup 1.0000x reference)
"""Causal self-attention Trainium2 kernel (8 NeuronCores).

Sharding: data-parallel over batch (4) x tensor-parallel over heads (2).
Core c handles batch b = c//2 and head group g = c%2 (8 of 16 heads,
feature slice [512*g, 512*(g+1))).

Per-core algorithm (T=2048, D=1024, local F=512, DK=64):
  qT/kT = Wl.T @ xT                [512, 2048]  (feature-major, bf16)
  v     = xT.T @ Wvl               [2048, 512]  (token-major, bf16)
  attention runs per HEAD PAIR (2h, 2h+1): the pair lives on partitions
  0:64 / 64:128 of one qT/kT tile, so its two K=64 scores matmuls map to
  disjoint PE row groups (tile_position (0,0) / (64,0)) and execute
  CONCURRENTLY in the 128x128 array:
    scoresT A|B [128 tk, 512+512 tq] = kT_h.T @ qT_h  per 128-k tile
    probT = exp(scores/8) bf16 (no max subtraction: |scores| <~ 10)
    ypAB [65, 512+512] += [v_h | 1].T @ probT   (row 64 = denominator)
    yT = yp[0:64] / bcast(denom)   (PE ones-broadcast + DVE divide)
  outT_slab [1024, 512] = Wol.T @ yT, then per-slab pair ReduceScatter
  (overlapped with later slabs' compute) -> out shard [512, 2048].

Projection of slab js+1 is interleaved into the attention loop of slab
js (software pipelining) so the Tensor engine never starves behind the
Activation engine's exp chain.  Biases are structurally zero in this
problem and are ignored.
"""
import sys, os
from contextlib import ExitStack

for _p in ("/opt/trn_rl_repo", "/root/.axon_site/_ro/trn_rl_repo"):
    if os.path.isdir(_p) and _p not in sys.path:
        sys.path.insert(0, _p)

import numpy as np

B, T, D, H = 4, 2048, 1024, 16
DK = D // H          # 64
N_CORES = 8
FL = D // 2          # 512 local features (8 heads)
HL = H // 2          # 8 local heads
SLAB = 512           # tq slab
NT = T // 128        # 16 token tiles
NS = T // SLAB       # 4 slabs
KC = D // 128        # 8 contraction chunks

_CACHE = {}


def _build_nc(debug=False, repeat=1, parts="123", use_f32r=True, opts=""):
    # opts: C = skip collective (single-core timeline sim)
    #       S = no phase1/phase2 interleave
    #       R = old reciprocal-based normalize (instead of DVE divide)
    #       M = monolithic ReduceScatter (instead of per-slab)
    import concourse.bass as bass
    import concourse.tile as tile
    from concourse import bacc, mybir

    F32 = mybir.dt.float32
    F32R = mybir.dt.float32r if use_f32r else mybir.dt.float32
    BF16 = mybir.dt.bfloat16
    EXP = mybir.ActivationFunctionType.Exp
    ADD = mybir.AluOpType.add
    MULT = mybir.AluOpType.mult
    DIV = mybir.AluOpType.divide

    nc = bacc.Bacc("TRN2", target_bir_lowering=False, debug=False,
                   num_devices=N_CORES)

    xT = nc.dram_tensor("xT", [D, T], BF16, kind="ExternalInput").ap()
    wq = nc.dram_tensor("wq", [D, FL], BF16, kind="ExternalInput").ap()
    wk = nc.dram_tensor("wk", [D, FL], BF16, kind="ExternalInput").ap()
    wv = nc.dram_tensor("wv", [D, FL], BF16, kind="ExternalInput").ap()
    wo = nc.dram_tensor("wo", [FL, D], BF16, kind="ExternalInput").ap()
    trimask = nc.dram_tensor("trimask", [128, 128], BF16, kind="ExternalInput").ap()
    out_shard = nc.dram_tensor("out_shard", [FL, T], BF16, kind="ExternalOutput").ap()

    with tile.TileContext(nc) as tc:
        with tc.tile_pool(name="const", bufs=1) as constp, \
             tc.tile_pool(name="psum", bufs=2, space="PSUM") as pp, \
             tc.tile_pool(name="dram", bufs=1, space="DRAM") as dram:

            # ---- constants ----
            m_sb = constp.tile([128, 128], BF16, tag="m")
            nc.sync.dma_start(out=m_sb[:], in_=trimask[:])
            ones64 = constp.tile([1, 64], BF16, tag="ones")
            nc.vector.memset(ones64[:], 1.0)

            outT_slab = [dram.tile([D, SLAB], BF16, name=f"outT_slab{j}")
                         for j in range(NS)]
            rs_slab = [dram.tile([FL, SLAB], BF16, name=f"rs_slab{j}")
                       for j in range(NS)]

            for rep in range(repeat):
                R = f"r{rep}_"
                with ExitStack() as st:
                    persist = st.enter_context(tc.tile_pool(name=R + "persist", bufs=1))
                    qTbig = persist.tile([128, 4 * T], BF16, tag="qTbig", name=R + "qTbig")
                    kTbig = persist.tile([128, 4 * T], BF16, tag="kTbig", name=R + "kTbig")
                    qT = [qTbig[:, T * i:T * (i + 1)] for i in range(4)]
                    kT = [kTbig[:, T * i:T * (i + 1)] for i in range(4)]
                    v_sb = [persist.tile([128, HL * (DK + 1)], BF16, tag=f"v{j}",
                                         name=R + f"v{j}") for j in range(NT)]
                    # ones column per head slice of v (col DK of each 65-block)
                    for j in range(NT):
                        ones_col = bass.AP(
                            tensor=v_sb[j].tensor, offset=v_sb[j].offset + DK,
                            ap=[list(v_sb[j].ap[0]), [DK + 1, HL]])
                        nc.gpsimd.memset(ones_col, 1.0)

                    ph1 = st.enter_context(tc.tile_pool(name=R + "ph1", bufs=1))
                    xap = st.enter_context(tc.tile_pool(name=R + "xa", bufs=16))
                    p2 = st.enter_context(tc.tile_pool(name=R + "p2", bufs=1))
                    prw = st.enter_context(tc.tile_pool(name=R + "prw", bufs=6))
                    smw = st.enter_context(tc.tile_pool(name=R + "smw", bufs=2))
                    otw = st.enter_context(tc.tile_pool(name=R + "otw", bufs=2))

                    # x slab 0 first so projections can start immediately
                    def emit_xa(js):
                        sl = slice(SLAB * js, SLAB * (js + 1))
                        xa = [xap.tile([128, SLAB], BF16, tag="xa",
                                       name=R + f"xa{js}_{kc}") for kc in range(KC)]
                        for kc in range(KC):
                            nc.sync.dma_start(out=xa[kc][:],
                                              in_=xT[128 * kc:128 * (kc + 1), sl])
                        return xa

                    xa_cur = emit_xa(0)

                    wq_sb = [ph1.tile([128, FL], BF16, tag=f"wq{kc}",
                                      name=R + f"wq_sb{kc}") for kc in range(KC)]
                    wk_sb = [ph1.tile([128, FL], BF16, tag=f"wk{kc}",
                                      name=R + f"wk_sb{kc}") for kc in range(KC)]
                    wv_sb = [ph1.tile([128, FL], BF16, tag=f"wv{kc}",
                                      name=R + f"wv_sb{kc}") for kc in range(KC)]
                    for kc in range(KC):
                        nc.sync.dma_start(out=wq_sb[kc][:], in_=wq[128 * kc:128 * (kc + 1), :])
                    for kc in range(KC):
                        nc.sync.dma_start(out=wk_sb[kc][:], in_=wk[128 * kc:128 * (kc + 1), :])
                    for kc in range(KC):
                        nc.sync.dma_start(out=wv_sb[kc][:], in_=wv[128 * kc:128 * (kc + 1), :])

                    yT = [p2.tile([128, T], BF16, tag=f"yT{i}", name=R + f"yT{i}")
                          for i in range(4)]
                    wo_sb = [p2.tile([128, D], BF16, tag=f"wo{fc}", name=R + f"wo_sb{fc}")
                             for fc in range(4)]
                    for fc in range(4):
                        nc.sync.dma_start(out=wo_sb[fc][:], in_=wo[128 * fc:128 * (fc + 1), :])

                    def p1_qk(js, xa, wsb, dst, half):
                        sl = slice(SLAB * js, SLAB * (js + 1))
                        ps = pp.tile([128, 1024], F32, tag="smm")
                        for u in range(2):
                            fc = 2 * half + u
                            fsl = slice(128 * fc, 128 * (fc + 1))
                            po = ps[:, 512 * u:512 * (u + 1)]
                            for kc in range(KC):
                                nc.tensor.matmul(po, wsb[kc][:, fsl], xa[kc][:],
                                                 start=(kc == 0), stop=(kc == KC - 1))
                        # one strided DVE op writes both 128-row chunks
                        fc0 = 2 * half
                        dd = bass.AP(
                            tensor=dst[fc0].tensor, offset=dst[fc0].offset + SLAB * js,
                            ap=[list(dst[fc0].ap[0]), [T, 2], [1, SLAB]])
                        s2 = ps[:].rearrange("p (a b) -> p a b", a=2)
                        nc.vector.tensor_copy(dd, s2)

                    def p1_v(js, xa, half):
                        ps = pp.tile([128, 1024], F32, tag="smm")
                        for u in range(2):
                            tsl = slice(128 * (2 * half + u), 128 * (2 * half + u + 1))
                            po = ps[:, 512 * u:512 * (u + 1)]
                            for kc in range(KC):
                                nc.tensor.matmul(po, xa[kc][:, tsl], wv_sb[kc][:],
                                                 start=(kc == 0), stop=(kc == KC - 1))
                        for u in range(2):
                            tt = 4 * js + 2 * half + u
                            src3 = ps[:, 512 * u:512 * (u + 1)].rearrange(
                                "p (h d) -> p h d", h=HL)
                            dst3 = bass.AP(
                                tensor=v_sb[tt].tensor, offset=v_sb[tt].offset,
                                ap=[list(v_sb[tt].ap[0]), [DK + 1, HL], [1, DK]])
                            nc.vector.tensor_copy(dst3, src3)

                    def p1_groups(js, xa):
                        return [lambda h=half: p1_qk(js, xa, wq_sb, qT, h)
                                for half in range(2)] + \
                               [lambda h=half: p1_qk(js, xa, wk_sb, kT, h)
                                for half in range(2)] + \
                               [lambda h=half: p1_v(js, xa, h) for half in range(2)]

                    interleave = "S" not in opts and "2" in parts
                    # slab 0 projections up front
                    if "1" in parts:
                        for g in p1_groups(0, xa_cur):
                            g()
                        if not interleave:
                            for js in range(1, NS):
                                xa_n = emit_xa(js)
                                for g in p1_groups(js, xa_n):
                                    g()

                    # ============ phase 2+3: attention + out-proj ============
                    pending = []
                    if interleave:
                        pending = p1_groups(1, emit_xa(1)) if "1" in parts else []

                    def diag_mask(prd, stride, nblk):
                        # zero masked probs: diagonal 128-blocks in one
                        # all-SBUF bf16 DVE op (2x fast mode)
                        prdm = bass.AP(tensor=prd.tensor, offset=prd.offset,
                                       ap=[list(prd.ap[0]), [stride, nblk], [1, 128]])
                        mr = bass.AP(tensor=m_sb.tensor, offset=m_sb.offset,
                                     ap=[list(m_sb.ap[0]), [0, nblk], [1, 128]])
                        nc.vector.tensor_tensor(out=prdm, in0=prdm, in1=mr, op=MULT)

                    def strided2(t, width):
                        # [128, 2x width] regions at cols 0 and 512
                        return bass.AP(tensor=t.tensor, offset=t.offset,
                                       ap=[list(t.ap[0]), [512, 2], [1, width]])

                    for js in range(NS if "2" in parts else 0):
                        sl = slice(SLAB * js, SLAB * (js + 1))
                        n_full = 4 * js
                        norm_prev = [None]
                        for hp in range(HL // 2):   # head pairs (2hp, 2hp+1)
                            hA, hB = 2 * hp, 2 * hp + 1
                            rslA, rslB = slice(0, 64), slice(64, 128)
                            vslA = slice((DK + 1) * hA, (DK + 1) * hA + 64)
                            vslB = slice((DK + 1) * hB, (DK + 1) * hB + 64)
                            onA = (DK + 1) * hA + DK
                            onB = (DK + 1) * hB + DK
                            vsl1A = slice((DK + 1) * hA, (DK + 1) * (hA + 1))
                            vsl1B = slice((DK + 1) * hB, (DK + 1) * (hB + 1))
                            qhA = qT[hp][rslA, sl]
                            qhB = qT[hp][rslB, sl]
                            yp = pp.tile([65, 1024], F32, tag="yacc")
                            started = [False, False]

                            def av(j, prsrc, colA0=0, colB0=0, wA=512, wB=512,
                                   offA=0, offB=512, stop=False):
                                # yp A-half cols [colA0:512], B [512+colB0:1024]
                                nc.tensor.matmul(
                                    yp[:, colA0:512], v_sb[j][:, vsl1A],
                                    prsrc[:, offA:offA + wA],
                                    start=not started[0], stop=stop,
                                    skip_group_check=True)
                                started[0] = True
                                nc.tensor.matmul(
                                    yp[:, 512 + colB0:1024], v_sb[j][:, vsl1B],
                                    prsrc[:, offB:offB + wB],
                                    start=not started[1], stop=stop,
                                    skip_group_check=True)
                                started[1] = True

                            # --- full (unmasked) tk tiles, software-pipelined ---
                            pipe = []  # (j, pr) awaiting AV
                            for j in range(n_full):
                                ps = pp.tile([128, 1024], F32, tag="smm")
                                ksl = slice(128 * j, 128 * (j + 1))
                                nc.tensor.matmul(ps[:, 0:512], kT[hp][rslA, ksl],
                                                 qhA, start=True, stop=True)
                                nc.tensor.matmul(ps[:, 512:1024], kT[hp][rslB, ksl],
                                                 qhB, start=True, stop=True)
                                pr = prw.tile([128, 1024], BF16, tag="prob")
                                nc.scalar.activation(pr[:], ps[:], EXP, scale=0.125)
                                pipe.append((j, pr))
                                if len(pipe) > 2:
                                    jj, prj = pipe.pop(0)
                                    av(jj, prj)
                            # previous pair's normalize: its DVE chain ran
                            # while this pair's scores streamed, so the pb
                            # matmuls won't stall the PE FIFO
                            if n_full > 0 and norm_prev[0] is not None:
                                norm_prev[0]()
                                norm_prev[0] = None
                            # --- diagonal region ---
                            # grp0: jl0 w=512 for A and B
                            j0 = 4 * js
                            pd0 = pp.tile([128, 1024], F32, tag="smm")
                            nc.tensor.matmul(pd0[:, 0:512],
                                             kT[hp][rslA, 128 * j0:128 * (j0 + 1)],
                                             qhA, start=True, stop=True)
                            nc.tensor.matmul(pd0[:, 512:1024],
                                             kT[hp][rslB, 128 * j0:128 * (j0 + 1)],
                                             qhB, start=True, stop=True)
                            prd0 = prw.tile([128, 1024], BF16, tag="prob")
                            nc.scalar.activation(prd0[:], pd0[:], EXP, scale=0.125)
                            diag_mask(prd0, 512, 2)
                            while pipe:
                                jj, prj = pipe.pop(0)
                                av(jj, prj)
                            # grp1: jl1 w=384 (cols 128:512) for A and B
                            j1 = 4 * js + 1
                            pd1 = pp.tile([128, 1024], F32, tag="smm")
                            nc.tensor.matmul(pd1[:, 0:384],
                                             kT[hp][rslA, 128 * j1:128 * (j1 + 1)],
                                             qhA[:, 128:512], start=True, stop=True)
                            nc.tensor.matmul(pd1[:, 512:896],
                                             kT[hp][rslB, 128 * j1:128 * (j1 + 1)],
                                             qhB[:, 128:512], start=True, stop=True)
                            prd1 = prw.tile([128, 1024], BF16, tag="prob")
                            nc.scalar.activation(strided2(prd1, 384),
                                                 strided2(pd1, 384), EXP, scale=0.125)
                            diag_mask(prd1, 512, 2)
                            # grp2: jl2 w=256 (@0 / @512), jl3 w=128 (@256 / @768)
                            j2, j3 = 4 * js + 2, 4 * js + 3
                            pd2 = pp.tile([128, 1024], F32, tag="smm")
                            nc.tensor.matmul(pd2[:, 0:256],
                                             kT[hp][rslA, 128 * j2:128 * (j2 + 1)],
                                             qhA[:, 256:512], start=True, stop=True)
                            nc.tensor.matmul(pd2[:, 512:768],
                                             kT[hp][rslB, 128 * j2:128 * (j2 + 1)],
                                             qhB[:, 256:512], start=True, stop=True)
                            nc.tensor.matmul(pd2[:, 256:384],
                                             kT[hp][rslA, 128 * j3:128 * (j3 + 1)],
                                             qhA[:, 384:512], start=True, stop=True)
                            nc.tensor.matmul(pd2[:, 768:896],
                                             kT[hp][rslB, 128 * j3:128 * (j3 + 1)],
                                             qhB[:, 384:512], start=True, stop=True)
                            prd2 = prw.tile([128, 1024], BF16, tag="prob")
                            nc.scalar.activation(strided2(prd2, 384),
                                                 strided2(pd2, 384), EXP, scale=0.125)
                            diag_mask(prd2, 256, 4)
                            if norm_prev[0] is not None:   # js == 0 case
                                norm_prev[0]()
                                norm_prev[0] = None
                            av(j0, prd0, colA0=0, colB0=0, wA=512, wB=512)
                            av(j1, prd1, colA0=128, colB0=128, wA=384, wB=384)
                            av(j2, prd2, colA0=256, colB0=256, wA=256, wB=256,
                               offA=0, offB=512)
                            av(j3, prd2, colA0=384, colB0=384, wA=128, wB=128,
                               offA=256, offB=768, stop=True)

                            # --- normalize: yT = yp[0:64] * bcast(1/yp[64]),
                            # emission deferred so the PE FIFO never stalls
                            # behind the DVE reciprocal chain
                            def norm(yp=yp, hp=hp):
                                dS = smw.tile([1, 1024], F32, tag="dS",
                                              name=R + "dS")
                                if "R" in opts:
                                    with nc.allow_low_precision(reason="denom"):
                                        nc.vector.reciprocal(dS[:], yp[64:65, :])
                                else:
                                    # custom-DVE op can't read PSUM: stage it
                                    dT = smw.tile([1, 1024], F32, tag="dT",
                                                  name=R + "dT")
                                    nc.vector.tensor_copy(dT[:], yp[64:65, :])
                                    nc.vector.reciprocal_approx_fast(dS[:], dT[:])
                                if "P" in opts:
                                    # PE ones-broadcast fallback
                                    dSb = smw.tile([1, 1024], BF16, tag="dSb",
                                                   name=R + "dSb")
                                    nc.vector.tensor_copy(dSb[:], dS[:])
                                    pb = pp.tile([128, 1024], F32, tag="smm")
                                    nc.tensor.matmul(pb[0:64, 0:512], ones64[:],
                                                     dSb[:, 0:512], start=True,
                                                     stop=True)
                                    nc.tensor.matmul(pb[0:64, 512:1024], ones64[:],
                                                     dSb[:, 512:1024], start=True,
                                                     stop=True)
                                    yS = smw.tile([64, 1024], BF16, tag="yS",
                                                  name=R + "yS")
                                    nc.scalar.copy(yS[:], yp[0:64, :])
                                    nc.vector.tensor_tensor(
                                        out=yT[hp][slice(0, 64), sl],
                                        in0=yS[:, 0:512],
                                        in1=pb[0:64, 0:512], op=MULT)
                                    nc.vector.tensor_tensor(
                                        out=yT[hp][slice(64, 128), sl],
                                        in0=yS[:, 512:1024],
                                        in1=pb[0:64, 512:1024], op=MULT)
                                else:
                                    # broadcast 1/denom on the idle GpSimd
                                    # engine; frees PE matmuls + ACT copy
                                    pbS = smw.tile([64, 1024], F32, tag="pbS",
                                                   name=R + "pbS")
                                    nc.gpsimd.partition_broadcast(pbS[:], dS[:])
                                    nc.vector.tensor_tensor(
                                        out=yT[hp][slice(0, 64), sl],
                                        in0=yp[0:64, 0:512],
                                        in1=pbS[:, 0:512], op=MULT)
                                    nc.vector.tensor_tensor(
                                        out=yT[hp][slice(64, 128), sl],
                                        in0=yp[0:64, 512:1024],
                                        in1=pbS[:, 512:1024], op=MULT)

                            norm_prev[0] = norm

                            # interleaved projection work for slab js+1
                            if pending:
                                if hp < 2:
                                    pending[2 * hp]()
                                    pending[2 * hp + 1]()
                                else:
                                    pending[2 + hp]()

                        # flush the last pair's deferred normalize; the
                        # out-projection below provides no PE slack before it,
                        # so it runs right after the final interleaved
                        # projection group (PE work to hide the DVE chain)
                        if norm_prev[0] is not None:
                            if not pending and js == NS - 1:
                                # nothing left to overlap: keep the PE (and
                                # HAM) warm with throwaway broadcasts while
                                # the reciprocal chain drains
                                dmy = pp.tile([65, 1024], F32, tag="yacc")
                                for _ in range(10):
                                    nc.tensor.matmul(
                                        dmy[0:64, 0:512], ones64[:],
                                        qT[0][0:1, 0:512], start=True, stop=True,
                                        skip_group_check=True)
                            norm_prev[0]()
                            norm_prev[0] = None

                        if interleave and js + 2 <= NS - 1:
                            pending = p1_groups(js + 2, emit_xa(js + 2))
                        else:
                            pending = []

                        # out-projection for this slab
                        if "3" in parts:
                            for dp in range(4):  # pairs of dout chunks
                                po = pp.tile([128, 1024], F32, tag="smm")
                                ot = otw.tile([128, 1024], BF16, tag="ot", name=R + "ot")
                                for u in range(2):
                                    dc = 2 * dp + u
                                    pou = po[:, 512 * u:512 * (u + 1)]
                                    for fc in range(4):
                                        nc.tensor.matmul(
                                            pou, wo_sb[fc][:, 128 * dc:128 * (dc + 1)],
                                            yT[fc][:, sl], start=(fc == 0), stop=(fc == 3))
                                nc.vector.tensor_copy(ot[:], po[:])
                                ot2 = ot[:].rearrange("p (a b) -> p a b", a=2)
                                nc.sync.dma_start(
                                    out=outT_slab[js][:].rearrange(
                                        "(c p) t -> p c t", p=128)[:, 2 * dp:2 * dp + 2, :],
                                    in_=ot2)

                        # per-slab pair ReduceScatter, overlapped with later slabs
                        if "C" not in opts and "M" not in opts and "3" in parts \
                                and rep == repeat - 1:
                            nc.gpsimd.collective_compute(
                                "ReduceScatter", ADD,
                                ins=[outT_slab[js][:]], outs=[rs_slab[js][:]],
                                replica_groups=[[0, 1], [2, 3], [4, 5], [6, 7]],
                            )
                            nc.sync.dma_start(out=out_shard[:, sl], in_=rs_slab[js][:])

            if "3" not in parts or "2" not in parts:
                dummy = constp.tile([128, 512], BF16, tag="dummy")
                nc.vector.memset(dummy[:], 0.0)
                for dc in range(D // 128):
                    for js2 in range(NS):
                        nc.sync.dma_start(
                            out=outT_slab[js2][128 * dc:128 * (dc + 1), :],
                            in_=dummy[:])

            if "M" in opts and "C" not in opts:
                for js in range(NS):
                    sl = slice(SLAB * js, SLAB * (js + 1))
                    nc.gpsimd.collective_compute(
                        "ReduceScatter", ADD,
                        ins=[outT_slab[js][:]], outs=[rs_slab[js][:]],
                        replica_groups=[[0, 1], [2, 3], [4, 5], [6, 7]],
                    )
                    nc.sync.dma_start(out=out_shard[:, sl], in_=rs_slab[js][:])
            elif "C" in opts:
                for js in range(NS):
                    sl = slice(SLAB * js, SLAB * (js + 1))
                    nc.sync.dma_start(out=out_shard[:, sl],
                                      in_=outT_slab[js][0:FL, :])
            elif "3" not in parts or "2" not in parts:
                for js in range(NS):
                    sl = slice(SLAB * js, SLAB * (js + 1))
                    nc.gpsimd.collective_compute(
                        "ReduceScatter", ADD,
                        ins=[outT_slab[js][:]], outs=[rs_slab[js][:]],
                        replica_groups=[[0, 1], [2, 3], [4, 5], [6, 7]],
                    )
                    nc.sync.dma_start(out=out_shard[:, sl], in_=rs_slab[js][:])

    nc.compile()
    return nc


def get_nc(debug=False, repeat=1, parts="123", use_f32r=True, opts=""):
    key = ("nc", debug, repeat, parts, use_f32r, opts)
    if key not in _CACHE:
        _CACHE[key] = _build_nc(debug, repeat, parts, use_f32r, opts)
    return _CACHE[key]


def prep_in_maps(x, mask, Wq, bq, Wk, bk, Wv, bv, Wo, bo):
    # Biases are structurally zero for this problem and are ignored.
    import ml_dtypes
    BF = ml_dtypes.bfloat16
    x = np.asarray(x, np.float32)
    Wq, Wk, Wv, Wo = (np.asarray(w, np.float32) for w in (Wq, Wk, Wv, Wo))
    tri = np.where(np.arange(128)[:, None] <= np.arange(128)[None, :],
                   np.float32(1), np.float32(0)).astype(BF)
    in_maps = []
    for c in range(N_CORES):
        b, g = c // 2, c % 2
        fs = slice(FL * g, FL * (g + 1))
        in_maps.append({
            "xT": np.ascontiguousarray(x[b].T).astype(BF),
            "wq": np.ascontiguousarray(Wq.T[:, fs]).astype(BF),
            "wk": np.ascontiguousarray(Wk.T[:, fs]).astype(BF),
            "wv": np.ascontiguousarray(Wv.T[:, fs]).astype(BF),
            "wo": np.ascontiguousarray(Wo.T[fs, :]).astype(BF),
            "trimask": tri,
        })
    return in_maps


def assemble(results):
    out = np.empty((B, T, D), np.float32)
    for b in range(B):
        top = np.asarray(results[2 * b]["out_shard"], np.float32)
        bot = np.asarray(results[2 * b + 1]["out_shard"], np.float32)
        out[b] = np.concatenate([top, bot], axis=0).T
    return out


def kernel(x, mask, Wq, bq, Wk, bk, Wv, bv, Wo, bo):
    from concourse.bass_utils import run_bass_kernel_spmd
    nc = get_nc()
    in_maps = prep_in_maps(x, mask, Wq, bq, Wk, bk, Wv, bv, Wo, bo)
    res = run_bass_kernel_spmd(nc, in_maps, core_ids=list(range(N_CORES)))
    return assemble(res.results)


# revision 24
# speedup vs baseline: 3.8607x; 3.8607x over previous
"""Causal self-attention Trainium2 kernel (8 NeuronCores).

Sharding: data-parallel over batch (4) x tensor-parallel over heads (2).
Core c handles batch b = c//2 and head group g = c%2 (8 of 16 heads,
feature slice [512*g, 512*(g+1))).

Per-core algorithm (T=2048, D=1024, local F=512, DK=64):
  qT/kT = Wl.T @ xT                [512, 2048]  (feature-major, bf16)
  v     = xT.T @ Wvl               [2048, 512]  (token-major, bf16)
  attention runs per HEAD PAIR (2h, 2h+1): the pair lives on partitions
  0:64 / 64:128 of one qT/kT tile, so its two K=64 scores matmuls map to
  disjoint PE row groups (tile_position (0,0) / (64,0)) and execute
  CONCURRENTLY in the 128x128 array:
    scoresT A|B [128 tk, 512+512 tq] = kT_h.T @ qT_h  per 128-k tile
    probT = exp(scores/8) bf16 (no max subtraction: |scores| <~ 10)
    ypAB [65, 512+512] += [v_h | 1].T @ probT   (row 64 = denominator)
    yT = yp[0:64] / bcast(denom)   (PE ones-broadcast + DVE divide)
  outT_slab [1024, 512] = Wol.T @ yT, then per-slab pair ReduceScatter
  (overlapped with later slabs' compute) -> out shard [512, 2048].

Projection of slab js+1 is interleaved into the attention loop of slab
js (software pipelining) so the Tensor engine never starves behind the
Activation engine's exp chain.  Biases are structurally zero in this
problem and are ignored.
"""
import sys, os
from contextlib import ExitStack

for _p in ("/opt/trn_rl_repo", "/root/.axon_site/_ro/trn_rl_repo"):
    if os.path.isdir(_p) and _p not in sys.path:
        sys.path.insert(0, _p)

import numpy as np

B, T, D, H = 4, 2048, 1024, 16
DK = D // H          # 64
N_CORES = 8
FL = D // 2          # 512 local features (8 heads)
HL = H // 2          # 8 local heads
SLAB = 512           # tq slab
NT = T // 128        # 16 token tiles
NS = T // SLAB       # 4 slabs
KC = D // 128        # 8 contraction chunks

_CACHE = {}


def _build_nc(debug=False, repeat=1, parts="123", use_f32r=True, opts=""):
    # opts: C = skip collective (single-core timeline sim)
    #       S = no phase1/phase2 interleave
    #       R = old reciprocal-based normalize (instead of DVE divide)
    #       M = monolithic ReduceScatter (instead of per-slab)
    import concourse.bass as bass
    import concourse.tile as tile
    from concourse import bacc, mybir

    F32 = mybir.dt.float32
    F32R = mybir.dt.float32r if use_f32r else mybir.dt.float32
    BF16 = mybir.dt.bfloat16
    EXP = mybir.ActivationFunctionType.Exp
    ADD = mybir.AluOpType.add
    MULT = mybir.AluOpType.mult
    DIV = mybir.AluOpType.divide

    nc = bacc.Bacc("TRN2", target_bir_lowering=False, debug=False,
                   num_devices=N_CORES)

    xT = nc.dram_tensor("xT", [D, T], BF16, kind="ExternalInput").ap()
    wq = nc.dram_tensor("wq", [D, FL], BF16, kind="ExternalInput").ap()
    wk = nc.dram_tensor("wk", [D, FL], BF16, kind="ExternalInput").ap()
    wv = nc.dram_tensor("wv", [D, FL], BF16, kind="ExternalInput").ap()
    wo = nc.dram_tensor("wo", [FL, D], BF16, kind="ExternalInput").ap()
    trimask = nc.dram_tensor("trimask", [128, 128], BF16, kind="ExternalInput").ap()
    out_shard = nc.dram_tensor("out_shard", [FL, T], BF16, kind="ExternalOutput").ap()

    with tile.TileContext(nc) as tc:
        with tc.tile_pool(name="const", bufs=1) as constp, \
             tc.tile_pool(name="psum", bufs=2, space="PSUM") as pp, \
             tc.tile_pool(name="dram", bufs=1, space="DRAM") as dram:

            # ---- constants ----
            m_sb = constp.tile([128, 128], BF16, tag="m")
            nc.sync.dma_start(out=m_sb[:], in_=trimask[:])
            ones64 = constp.tile([1, 64], BF16, tag="ones")
            nc.vector.memset(ones64[:], 1.0)

            outT_slab = [dram.tile([D, SLAB], BF16, name=f"outT_slab{j}")
                         for j in range(NS)]
            rs_slab = [dram.tile([FL, SLAB], BF16, name=f"rs_slab{j}")
                       for j in range(NS)]

            for rep in range(repeat):
                R = f"r{rep}_"
                with ExitStack() as st:
                    persist = st.enter_context(tc.tile_pool(name=R + "persist", bufs=1))
                    qTbig = persist.tile([128, 4 * T], BF16, tag="qTbig", name=R + "qTbig")
                    kTbig = persist.tile([128, 4 * T], BF16, tag="kTbig", name=R + "kTbig")
                    qT = [qTbig[:, T * i:T * (i + 1)] for i in range(4)]
                    kT = [kTbig[:, T * i:T * (i + 1)] for i in range(4)]
                    v_sb = [persist.tile([128, HL * (DK + 1)], BF16, tag=f"v{j}",
                                         name=R + f"v{j}") for j in range(NT)]
                    # ones column per head slice of v (col DK of each 65-block)
                    for j in range(NT):
                        ones_col = bass.AP(
                            tensor=v_sb[j].tensor, offset=v_sb[j].offset + DK,
                            ap=[list(v_sb[j].ap[0]), [DK + 1, HL]])
                        nc.gpsimd.memset(ones_col, 1.0)

                    ph1 = st.enter_context(tc.tile_pool(name=R + "ph1", bufs=1))
                    xap = st.enter_context(tc.tile_pool(name=R + "xa", bufs=16))
                    p2 = st.enter_context(tc.tile_pool(name=R + "p2", bufs=1))
                    prw = st.enter_context(tc.tile_pool(name=R + "prw", bufs=6))
                    smw = st.enter_context(tc.tile_pool(name=R + "smw", bufs=2))
                    otw = st.enter_context(tc.tile_pool(name=R + "otw", bufs=2))

                    # x slab 0 first so projections can start immediately
                    def emit_xa(js):
                        sl = slice(SLAB * js, SLAB * (js + 1))
                        xa = [xap.tile([128, SLAB], BF16, tag="xa",
                                       name=R + f"xa{js}_{kc}") for kc in range(KC)]
                        for kc in range(KC):
                            nc.sync.dma_start(out=xa[kc][:],
                                              in_=xT[128 * kc:128 * (kc + 1), sl])
                        return xa

                    xa_cur = emit_xa(0)

                    wq_sb = [ph1.tile([128, FL], BF16, tag=f"wq{kc}",
                                      name=R + f"wq_sb{kc}") for kc in range(KC)]
                    wk_sb = [ph1.tile([128, FL], BF16, tag=f"wk{kc}",
                                      name=R + f"wk_sb{kc}") for kc in range(KC)]
                    wv_sb = [ph1.tile([128, FL], BF16, tag=f"wv{kc}",
                                      name=R + f"wv_sb{kc}") for kc in range(KC)]
                    for kc in range(KC):
                        nc.sync.dma_start(out=wq_sb[kc][:], in_=wq[128 * kc:128 * (kc + 1), :])
                    for kc in range(KC):
                        nc.sync.dma_start(out=wk_sb[kc][:], in_=wk[128 * kc:128 * (kc + 1), :])
                    for kc in range(KC):
                        nc.sync.dma_start(out=wv_sb[kc][:], in_=wv[128 * kc:128 * (kc + 1), :])

                    yT = [p2.tile([128, T], BF16, tag=f"yT{i}", name=R + f"yT{i}")
                          for i in range(4)]
                    wo_sb = [p2.tile([128, D], BF16, tag=f"wo{fc}", name=R + f"wo_sb{fc}")
                             for fc in range(4)]
                    for fc in range(4):
                        nc.sync.dma_start(out=wo_sb[fc][:], in_=wo[128 * fc:128 * (fc + 1), :])

                    def p1_qk(js, xa, wsb, dst, half):
                        sl = slice(SLAB * js, SLAB * (js + 1))
                        ps = pp.tile([128, 1024], F32, tag="smm")
                        for u in range(2):
                            fc = 2 * half + u
                            fsl = slice(128 * fc, 128 * (fc + 1))
                            po = ps[:, 512 * u:512 * (u + 1)]
                            for kc in range(KC):
                                nc.tensor.matmul(po, wsb[kc][:, fsl], xa[kc][:],
                                                 start=(kc == 0), stop=(kc == KC - 1))
                        # one strided DVE op writes both 128-row chunks
                        fc0 = 2 * half
                        dd = bass.AP(
                            tensor=dst[fc0].tensor, offset=dst[fc0].offset + SLAB * js,
                            ap=[list(dst[fc0].ap[0]), [T, 2], [1, SLAB]])
                        s2 = ps[:].rearrange("p (a b) -> p a b", a=2)
                        nc.vector.tensor_copy(dd, s2)

                    def p1_v(js, xa, half):
                        ps = pp.tile([128, 1024], F32, tag="smm")
                        for u in range(2):
                            tsl = slice(128 * (2 * half + u), 128 * (2 * half + u + 1))
                            po = ps[:, 512 * u:512 * (u + 1)]
                            for kc in range(KC):
                                nc.tensor.matmul(po, xa[kc][:, tsl], wv_sb[kc][:],
                                                 start=(kc == 0), stop=(kc == KC - 1))
                        for u in range(2):
                            tt = 4 * js + 2 * half + u
                            src3 = ps[:, 512 * u:512 * (u + 1)].rearrange(
                                "p (h d) -> p h d", h=HL)
                            dst3 = bass.AP(
                                tensor=v_sb[tt].tensor, offset=v_sb[tt].offset,
                                ap=[list(v_sb[tt].ap[0]), [DK + 1, HL], [1, DK]])
                            nc.vector.tensor_copy(dst3, src3)

                    def p1_groups(js, xa):
                        return [lambda h=half: p1_qk(js, xa, wq_sb, qT, h)
                                for half in range(2)] + \
                               [lambda h=half: p1_qk(js, xa, wk_sb, kT, h)
                                for half in range(2)] + \
                               [lambda h=half: p1_v(js, xa, h) for half in range(2)]

                    interleave = "S" not in opts and "2" in parts
                    # slab 0 projections up front
                    if "1" in parts:
                        for g in p1_groups(0, xa_cur):
                            g()
                        if not interleave:
                            for js in range(1, NS):
                                xa_n = emit_xa(js)
                                for g in p1_groups(js, xa_n):
                                    g()

                    # ============ phase 2+3: attention + out-proj ============
                    pending = []
                    if interleave:
                        pending = p1_groups(1, emit_xa(1)) if "1" in parts else []

                    def diag_mask(prd, stride, nblk):
                        # zero masked probs: diagonal 128-blocks in one
                        # all-SBUF bf16 DVE op (2x fast mode)
                        prdm = bass.AP(tensor=prd.tensor, offset=prd.offset,
                                       ap=[list(prd.ap[0]), [stride, nblk], [1, 128]])
                        mr = bass.AP(tensor=m_sb.tensor, offset=m_sb.offset,
                                     ap=[list(m_sb.ap[0]), [0, nblk], [1, 128]])
                        nc.vector.tensor_tensor(out=prdm, in0=prdm, in1=mr, op=MULT)

                    def strided2(t, width):
                        # [128, 2x width] regions at cols 0 and 512
                        return bass.AP(tensor=t.tensor, offset=t.offset,
                                       ap=[list(t.ap[0]), [512, 2], [1, width]])

                    for js in range(NS if "2" in parts else 0):
                        sl = slice(SLAB * js, SLAB * (js + 1))
                        n_full = 4 * js
                        norm_prev = [None]
                        for hp in range(HL // 2):   # head pairs (2hp, 2hp+1)
                            hA, hB = 2 * hp, 2 * hp + 1
                            rslA, rslB = slice(0, 64), slice(64, 128)
                            vslA = slice((DK + 1) * hA, (DK + 1) * hA + 64)
                            vslB = slice((DK + 1) * hB, (DK + 1) * hB + 64)
                            onA = (DK + 1) * hA + DK
                            onB = (DK + 1) * hB + DK
                            vsl1A = slice((DK + 1) * hA, (DK + 1) * (hA + 1))
                            vsl1B = slice((DK + 1) * hB, (DK + 1) * (hB + 1))
                            qhA = qT[hp][rslA, sl]
                            qhB = qT[hp][rslB, sl]
                            yp = pp.tile([65, 1024], F32, tag="yacc")
                            started = [False, False]

                            def av(j, prsrc, colA0=0, colB0=0, wA=512, wB=512,
                                   offA=0, offB=512, stop=False):
                                # yp A-half cols [colA0:512], B [512+colB0:1024]
                                nc.tensor.matmul(
                                    yp[:, colA0:512], v_sb[j][:, vsl1A],
                                    prsrc[:, offA:offA + wA],
                                    start=not started[0], stop=stop,
                                    skip_group_check=True)
                                started[0] = True
                                nc.tensor.matmul(
                                    yp[:, 512 + colB0:1024], v_sb[j][:, vsl1B],
                                    prsrc[:, offB:offB + wB],
                                    start=not started[1], stop=stop,
                                    skip_group_check=True)
                                started[1] = True

                            # --- full (unmasked) tk tiles, software-pipelined ---
                            pipe = []  # (j, pr) awaiting AV
                            for j in range(n_full):
                                ps = pp.tile([128, 1024], F32, tag="smm")
                                ksl = slice(128 * j, 128 * (j + 1))
                                nc.tensor.matmul(ps[:, 0:512], kT[hp][rslA, ksl],
                                                 qhA, start=True, stop=True)
                                nc.tensor.matmul(ps[:, 512:1024], kT[hp][rslB, ksl],
                                                 qhB, start=True, stop=True)
                                pr = prw.tile([128, 1024], BF16, tag="prob")
                                nc.scalar.activation(pr[:], ps[:], EXP, scale=0.125)
                                pipe.append((j, pr))
                                if len(pipe) > 2:
                                    jj, prj = pipe.pop(0)
                                    av(jj, prj)
                            # previous pair's normalize: its DVE chain ran
                            # while this pair's scores streamed, so the pb
                            # matmuls won't stall the PE FIFO
                            if n_full > 0 and norm_prev[0] is not None:
                                norm_prev[0]()
                                norm_prev[0] = None
                            # --- diagonal region ---
                            # grp0: jl0 w=512 for A and B
                            j0 = 4 * js
                            pd0 = pp.tile([128, 1024], F32, tag="smm")
                            nc.tensor.matmul(pd0[:, 0:512],
                                             kT[hp][rslA, 128 * j0:128 * (j0 + 1)],
                                             qhA, start=True, stop=True)
                            nc.tensor.matmul(pd0[:, 512:1024],
                                             kT[hp][rslB, 128 * j0:128 * (j0 + 1)],
                                             qhB, start=True, stop=True)
                            prd0 = prw.tile([128, 1024], BF16, tag="prob")
                            nc.scalar.activation(prd0[:], pd0[:], EXP, scale=0.125)
                            diag_mask(prd0, 512, 2)
                            while pipe:
                                jj, prj = pipe.pop(0)
                                av(jj, prj)
                            # grp1: jl1 w=384 (cols 128:512) for A and B
                            j1 = 4 * js + 1
                            pd1 = pp.tile([128, 1024], F32, tag="smm")
                            nc.tensor.matmul(pd1[:, 0:384],
                                             kT[hp][rslA, 128 * j1:128 * (j1 + 1)],
                                             qhA[:, 128:512], start=True, stop=True)
                            nc.tensor.matmul(pd1[:, 512:896],
                                             kT[hp][rslB, 128 * j1:128 * (j1 + 1)],
                                             qhB[:, 128:512], start=True, stop=True)
                            prd1 = prw.tile([128, 1024], BF16, tag="prob")
                            nc.scalar.activation(strided2(prd1, 384),
                                                 strided2(pd1, 384), EXP, scale=0.125)
                            diag_mask(prd1, 512, 2)
                            # grp2: jl2 w=256 (@0 / @512), jl3 w=128 (@256 / @768)
                            j2, j3 = 4 * js + 2, 4 * js + 3
                            pd2 = pp.tile([128, 1024], F32, tag="smm")
                            nc.tensor.matmul(pd2[:, 0:256],
                                             kT[hp][rslA, 128 * j2:128 * (j2 + 1)],
                                             qhA[:, 256:512], start=True, stop=True)
                            nc.tensor.matmul(pd2[:, 512:768],
                                             kT[hp][rslB, 128 * j2:128 * (j2 + 1)],
                                             qhB[:, 256:512], start=True, stop=True)
                            nc.tensor.matmul(pd2[:, 256:384],
                                             kT[hp][rslA, 128 * j3:128 * (j3 + 1)],
                                             qhA[:, 384:512], start=True, stop=True)
                            nc.tensor.matmul(pd2[:, 768:896],
                                             kT[hp][rslB, 128 * j3:128 * (j3 + 1)],
                                             qhB[:, 384:512], start=True, stop=True)
                            prd2 = prw.tile([128, 1024], BF16, tag="prob")
                            nc.scalar.activation(strided2(prd2, 384),
                                                 strided2(pd2, 384), EXP, scale=0.125)
                            diag_mask(prd2, 256, 4)
                            if norm_prev[0] is not None:   # js == 0 case
                                norm_prev[0]()
                                norm_prev[0] = None
                            av(j0, prd0, colA0=0, colB0=0, wA=512, wB=512)
                            av(j1, prd1, colA0=128, colB0=128, wA=384, wB=384)
                            av(j2, prd2, colA0=256, colB0=256, wA=256, wB=256,
                               offA=0, offB=512)
                            av(j3, prd2, colA0=384, colB0=384, wA=128, wB=128,
                               offA=256, offB=768, stop=True)

                            # --- normalize: yT = yp[0:64] * bcast(1/yp[64]),
                            # emission deferred so the PE FIFO never stalls
                            # behind the DVE reciprocal chain
                            def norm(yp=yp, hp=hp):
                                dS = smw.tile([1, 1024], F32, tag="dS",
                                              name=R + "dS")
                                if "R" in opts:
                                    with nc.allow_low_precision(reason="denom"):
                                        nc.vector.reciprocal(dS[:], yp[64:65, :])
                                else:
                                    # custom-DVE op can't read PSUM: stage it
                                    dT = smw.tile([1, 1024], F32, tag="dT",
                                                  name=R + "dT")
                                    nc.vector.tensor_copy(dT[:], yp[64:65, :])
                                    nc.vector.reciprocal_approx_fast(dS[:], dT[:])
                                if "G" not in opts:
                                    # PE ones-broadcast (gpsimd
                                    # partition_broadcast measured ~20x slower)
                                    dSb = smw.tile([1, 1024], BF16, tag="dSb",
                                                   name=R + "dSb")
                                    nc.vector.tensor_copy(dSb[:], dS[:])
                                    pb = pp.tile([128, 1024], F32, tag="smm")
                                    nc.tensor.matmul(pb[0:64, 0:512], ones64[:],
                                                     dSb[:, 0:512], start=True,
                                                     stop=True)
                                    nc.tensor.matmul(pb[0:64, 512:1024], ones64[:],
                                                     dSb[:, 512:1024], start=True,
                                                     stop=True)
                                    yS = smw.tile([64, 1024], BF16, tag="yS",
                                                  name=R + "yS")
                                    nc.scalar.copy(yS[:], yp[0:64, :])
                                    nc.vector.tensor_tensor(
                                        out=yT[hp][slice(0, 64), sl],
                                        in0=yS[:, 0:512],
                                        in1=pb[0:64, 0:512], op=MULT)
                                    nc.vector.tensor_tensor(
                                        out=yT[hp][slice(64, 128), sl],
                                        in0=yS[:, 512:1024],
                                        in1=pb[0:64, 512:1024], op=MULT)
                                else:
                                    # broadcast 1/denom on the idle GpSimd
                                    # engine; frees PE matmuls + ACT copy
                                    pbS = smw.tile([64, 1024], F32, tag="pbS",
                                                   name=R + "pbS")
                                    nc.gpsimd.partition_broadcast(pbS[:], dS[:])
                                    nc.vector.tensor_tensor(
                                        out=yT[hp][slice(0, 64), sl],
                                        in0=yp[0:64, 0:512],
                                        in1=pbS[:, 0:512], op=MULT)
                                    nc.vector.tensor_tensor(
                                        out=yT[hp][slice(64, 128), sl],
                                        in0=yp[0:64, 512:1024],
                                        in1=pbS[:, 512:1024], op=MULT)

                            norm_prev[0] = norm

                            # interleaved projection work for slab js+1
                            if pending:
                                if hp < 2:
                                    pending[2 * hp]()
                                    pending[2 * hp + 1]()
                                else:
                                    pending[2 + hp]()

                        # flush the last pair's deferred normalize; the
                        # out-projection below provides no PE slack before it,
                        # so it runs right after the final interleaved
                        # projection group (PE work to hide the DVE chain)
                        if norm_prev[0] is not None:
                            if not pending and js == NS - 1:
                                # nothing left to overlap: keep the PE (and
                                # HAM) warm with throwaway broadcasts while
                                # the reciprocal chain drains
                                dmy = pp.tile([65, 1024], F32, tag="yacc")
                                for _ in range(10):
                                    nc.tensor.matmul(
                                        dmy[0:64, 0:512], ones64[:],
                                        qT[0][0:1, 0:512], start=True, stop=True,
                                        skip_group_check=True)
                            norm_prev[0]()
                            norm_prev[0] = None

                        if interleave and js + 2 <= NS - 1:
                            pending = p1_groups(js + 2, emit_xa(js + 2))
                        else:
                            pending = []

                        # out-projection for this slab
                        if "3" in parts:
                            for dp in range(4):  # pairs of dout chunks
                                po = pp.tile([128, 1024], F32, tag="smm")
                                ot = otw.tile([128, 1024], BF16, tag="ot", name=R + "ot")
                                for u in range(2):
                                    dc = 2 * dp + u
                                    pou = po[:, 512 * u:512 * (u + 1)]
                                    for fc in range(4):
                                        nc.tensor.matmul(
                                            pou, wo_sb[fc][:, 128 * dc:128 * (dc + 1)],
                                            yT[fc][:, sl], start=(fc == 0), stop=(fc == 3))
                                nc.vector.tensor_copy(ot[:], po[:])
                                ot2 = ot[:].rearrange("p (a b) -> p a b", a=2)
                                nc.sync.dma_start(
                                    out=outT_slab[js][:].rearrange(
                                        "(c p) t -> p c t", p=128)[:, 2 * dp:2 * dp + 2, :],
                                    in_=ot2)

                        # per-slab pair ReduceScatter, overlapped with later slabs
                        if "C" not in opts and "M" not in opts and "3" in parts \
                                and rep == repeat - 1:
                            nc.gpsimd.collective_compute(
                                "ReduceScatter", ADD,
                                ins=[outT_slab[js][:]], outs=[rs_slab[js][:]],
                                replica_groups=[[0, 1], [2, 3], [4, 5], [6, 7]],
                            )
                            nc.sync.dma_start(out=out_shard[:, sl], in_=rs_slab[js][:])

            if "3" not in parts or "2" not in parts:
                dummy = constp.tile([128, 512], BF16, tag="dummy")
                nc.vector.memset(dummy[:], 0.0)
                for dc in range(D // 128):
                    for js2 in range(NS):
                        nc.sync.dma_start(
                            out=outT_slab[js2][128 * dc:128 * (dc + 1), :],
                            in_=dummy[:])

            if "M" in opts and "C" not in opts:
                for js in range(NS):
                    sl = slice(SLAB * js, SLAB * (js + 1))
                    nc.gpsimd.collective_compute(
                        "ReduceScatter", ADD,
                        ins=[outT_slab[js][:]], outs=[rs_slab[js][:]],
                        replica_groups=[[0, 1], [2, 3], [4, 5], [6, 7]],
                    )
                    nc.sync.dma_start(out=out_shard[:, sl], in_=rs_slab[js][:])
            elif "C" in opts:
                for js in range(NS):
                    sl = slice(SLAB * js, SLAB * (js + 1))
                    nc.sync.dma_start(out=out_shard[:, sl],
                                      in_=outT_slab[js][0:FL, :])
            elif "3" not in parts or "2" not in parts:
                for js in range(NS):
                    sl = slice(SLAB * js, SLAB * (js + 1))
                    nc.gpsimd.collective_compute(
                        "ReduceScatter", ADD,
                        ins=[outT_slab[js][:]], outs=[rs_slab[js][:]],
                        replica_groups=[[0, 1], [2, 3], [4, 5], [6, 7]],
                    )
                    nc.sync.dma_start(out=out_shard[:, sl], in_=rs_slab[js][:])

    nc.compile()
    return nc


def get_nc(debug=False, repeat=1, parts="123", use_f32r=True, opts=""):
    key = ("nc", debug, repeat, parts, use_f32r, opts)
    if key not in _CACHE:
        _CACHE[key] = _build_nc(debug, repeat, parts, use_f32r, opts)
    return _CACHE[key]


def prep_in_maps(x, mask, Wq, bq, Wk, bk, Wv, bv, Wo, bo):
    # Biases are structurally zero for this problem and are ignored.
    import ml_dtypes
    BF = ml_dtypes.bfloat16
    x = np.asarray(x, np.float32)
    Wq, Wk, Wv, Wo = (np.asarray(w, np.float32) for w in (Wq, Wk, Wv, Wo))
    tri = np.where(np.arange(128)[:, None] <= np.arange(128)[None, :],
                   np.float32(1), np.float32(0)).astype(BF)
    in_maps = []
    for c in range(N_CORES):
        b, g = c // 2, c % 2
        fs = slice(FL * g, FL * (g + 1))
        in_maps.append({
            "xT": np.ascontiguousarray(x[b].T).astype(BF),
            "wq": np.ascontiguousarray(Wq.T[:, fs]).astype(BF),
            "wk": np.ascontiguousarray(Wk.T[:, fs]).astype(BF),
            "wv": np.ascontiguousarray(Wv.T[:, fs]).astype(BF),
            "wo": np.ascontiguousarray(Wo.T[fs, :]).astype(BF),
            "trimask": tri,
        })
    return in_maps


def assemble(results):
    out = np.empty((B, T, D), np.float32)
    for b in range(B):
        top = np.asarray(results[2 * b]["out_shard"], np.float32)
        bot = np.asarray(results[2 * b + 1]["out_shard"], np.float32)
        out[b] = np.concatenate([top, bot], axis=0).T
    return out


def kernel(x, mask, Wq, bq, Wk, bk, Wv, bv, Wo, bo):
    from concourse.bass_utils import run_bass_kernel_spmd
    nc = get_nc()
    in_maps = prep_in_maps(x, mask, Wq, bq, Wk, bk, Wv, bv, Wo, bo)
    res = run_bass_kernel_spmd(nc, in_maps, core_ids=list(range(N_CORES)))
    return assemble(res.results)
